# revision 28
# baseline (speedup 1.0000x reference)
"""Causal self-attention on 8 trn2 NeuronCores.

Sharding: core c = (batch b = c//2, head-group g = c%2). Each core computes
QKV projection for its 8 heads of its batch, causal flash-attention in a
transposed (S^T) layout, and a partial out-projection (its 512 rows of
w_out). Host sums the two partials per batch and adds b_out.

Design (v9):
- x, w_qkv, w_out are cast to bf16 and x transposed on the HOST; all
  matmuls run bf16 (fp32 PSUM accumulate) with FWL weight loads.
- Diagonal narrowing: for the 4 k-blocks of each q-chunk's diagonal
  512x512 block, S^T / exp / PV only cover queries >= block start
  (shift-packed so each head's valid region stays contiguous); the causal
  mask is a single [128,128] lower-tri bf16 multiply per diagonal block.
- Attention is processed per HEAD: one [128,1024] fp32 PSUM tile per
  (head, 2-k-block group) gives pipeline depth 2 in 4 banks; PV lags one
  group behind S/exp. Short q-chunks (<=4 groups/head) interleave head
  PAIRS so the S->exp->PV pipeline has twice the depth to fill.
- Normalization: copy the PV ones-column (softmax denominator) row to
  SBUF, DRAM-bounce it across 64 partitions, DVE fast-reciprocal on the
  [64,512] (reciprocal_approx_fast silently corrupts on HW for
  single-partition base-64 PSUM reads), then one DVE mul deferred to the
  next head so DMA latency never blocks the DVE queue head. The last two
  heads broadcast via a tiny PE matmul instead (no DMA latency at the
  kernel tail).
- Out-projection of q-chunk i is deferred two chunks and interleaved as
  PE filler into later attention (as are the next chunk's projections), at
  per-group granularity, so the PE never idles long enough for the HAM
  clock gate to re-throttle. Chunk-3's V-projection fills early q-chunk-3
  steps.
- Startup: weight tiles stream m-major in 128x128 pieces across all three
  DMA queues, and chunk-0's Q/K/V units interleave so every queue's
  arrivals feed the PE concurrently.
"""

import numpy as np

B = 4
T = 2048
C = 1024
HG = 512          # head channels per core (8 heads x 64)
Dh = 64
NHL = 8           # local heads per core
TCH = 512         # T-chunk (q-chunk) width
NTC = T // TCH    # 4
NCC = C // 128    # 8 contraction chunks for projections
NMT = HG // 128   # 4 row-tiles of q/k channels
VW = NHL * (Dh + 1)   # 520: V tiles with a ones column per head

_CACHE = {}
_DEBUG_DUMP = False


def _build_nc():
    import concourse.bass as bass
    import concourse.bacc as bacc
    import concourse.tile as tile
    import concourse.mybir as mybir

    f32 = mybir.dt.float32
    f32r = mybir.dt.float32r
    bf16 = mybir.dt.bfloat16
    AF = mybir.ActivationFunctionType

    nc = bacc.Bacc("TRN2", target_bir_lowering=False, debug=False,
                   enable_asserts=False)
    # x arrives pre-transposed and pre-cast: [C, T] bf16
    xT_d = nc.dram_tensor("xT", [C, T], bf16, kind="ExternalInput").ap()
    wq_d = nc.dram_tensor("wq", [C, HG], bf16, kind="ExternalInput").ap()
    wk_d = nc.dram_tensor("wk", [C, HG], bf16, kind="ExternalInput").ap()
    wv_d = nc.dram_tensor("wv", [C, HG], bf16, kind="ExternalInput").ap()
    bq_d = nc.dram_tensor("bq", [HG], f32, kind="ExternalInput").ap()
    bk_d = nc.dram_tensor("bk", [HG], f32, kind="ExternalInput").ap()
    bv_d = nc.dram_tensor("bv", [HG], f32, kind="ExternalInput").ap()
    wo_d = nc.dram_tensor("wo", [HG, C], bf16, kind="ExternalInput").ap()
    tr_d = nc.dram_tensor("tri", [128, 128], bf16, kind="ExternalInput").ap()
    on_d = nc.dram_tensor("ones", [1, 64], f32r, kind="ExternalInput").ap()
    vo_d = nc.dram_tensor("vones", [128, NHL], bf16, kind="ExternalInput").ap()
    y_d = nc.dram_tensor("y", [T, C], bf16, kind="ExternalOutput").ap()
    # DRAM bounce rows for the per-(chunk, head) denominator broadcast
    rb_d = nc.dram_tensor("rbscratch", [NTC * NHL, TCH], f32,
                          kind="Internal").ap()
    if _DEBUG_DUMP:
        ktd_d = nc.dram_tensor("ktdump", [NMT, 128, T], bf16,
                               kind="ExternalOutput").ap()
        vd_d = nc.dram_tensor("vdump", [T // 128, 128, VW], bf16,
                              kind="ExternalOutput").ap()
        otd_d = nc.dram_tensor("otdump", [NMT, 128, TCH], bf16,
                               kind="ExternalOutput").ap()

    def mm(out, lhsT, rhs, start, stop):
        nc.tensor.matmul(out, lhsT, rhs, start=start, stop=stop)

    with tile.TileContext(nc) as tc:
        with tc.tile_pool(name="wp", bufs=1) as wp, \
             tc.tile_pool(name="ktp", bufs=1) as ktp, \
             tc.tile_pool(name="vp", bufs=1) as vp, \
             tc.tile_pool(name="qtp", bufs=2) as qtp, \
             tc.tile_pool(name="xp", bufs=2) as xp, \
             tc.tile_pool(name="esp", bufs=6) as esp, \
             tc.tile_pool(name="rp", bufs=2) as rp, \
             tc.tile_pool(name="otp", bufs=3) as otp, \
             tc.tile_pool(name="yst", bufs=2) as yst, \
             tc.tile_pool(name="psM", bufs=2, space="PSUM") as psM, \
             tc.tile_pool(name="psS", bufs=2, space="PSUM") as psS, \
             tc.tile_pool(name="psO", bufs=2, space="PSUM") as psO:

            # ---- startup loads spread across the three DMA queues so the
            # first Q-projection matmul can start after ~0.5 MB ----
            xin0 = [xp.tile([128, TCH], bf16, name=f"xt{c}", tag=f"xt{c}")
                    for c in range(NCC)]
            for c in range(4):
                nc.sync.dma_start(out=xin0[c],
                                  in_=xT_d[c * 128:(c + 1) * 128, 0:TCH])
            for c in range(4, NCC):
                nc.gpsimd.dma_start(out=xin0[c],
                                    in_=xT_d[c * 128:(c + 1) * 128, 0:TCH])
            wq_sb = [wp.tile([128, HG], bf16, name=f"wq{c}", tag=f"wq{c}")
                     for c in range(NCC)]
            wk_sb = [wp.tile([128, HG], bf16, name=f"wk{c}", tag=f"wk{c}")
                     for c in range(NCC)]
            wv_sb = [wp.tile([128, HG], bf16, name=f"wv{c}", tag=f"wv{c}")
                     for c in range(NCC)]
            wo_sb = [wp.tile([128, C], bf16, name=f"wo{m}", tag=f"wo{m}")
                     for m in range(NMT)]
            # m-major 128x128 pieces so Q-proj m=0 starts after 0.125 MB
            for m in range(NMT):
                for c in range(NCC):
                    nc.scalar.dma_start(
                        out=wq_sb[c][:, m * 128:(m + 1) * 128],
                        in_=wq_d[c * 128:(c + 1) * 128, m * 128:(m + 1) * 128])
            for m in range(NMT):
                for c in range(NCC):
                    nc.sync.dma_start(
                        out=wk_sb[c][:, m * 128:(m + 1) * 128],
                        in_=wk_d[c * 128:(c + 1) * 128, m * 128:(m + 1) * 128])
            for c in range(NCC):
                nc.gpsimd.dma_start(out=wv_sb[c],
                                    in_=wv_d[c * 128:(c + 1) * 128, :])
            for m in range(NMT):
                nc.scalar.dma_start(out=wo_sb[m],
                                    in_=wo_d[m * 128:(m + 1) * 128, :])
            bq_sb = [wp.tile([128, 1], f32, name=f"bq{m}", tag=f"bq{m}")
                     for m in range(NMT)]
            bk_sb = [wp.tile([128, 1], f32, name=f"bk{m}", tag=f"bk{m}")
                     for m in range(NMT)]
            for m in range(NMT):
                nc.gpsimd.dma_start(
                    out=bq_sb[m],
                    in_=bq_d[m * 128:(m + 1) * 128].rearrange("(p o) -> p o", o=1))
                nc.gpsimd.dma_start(
                    out=bk_sb[m],
                    in_=bk_d[m * 128:(m + 1) * 128].rearrange("(p o) -> p o", o=1))
            bv_bc = wp.tile([128, HG], f32, tag="bvbc")
            bv_src = bass.AP(tensor=bv_d.tensor, offset=bv_d.offset,
                             ap=[[0, 128]] + list(bv_d.ap))
            nc.gpsimd.dma_start(out=bv_bc, in_=bv_src)
            tri = wp.tile([128, 128], bf16, tag="tri")
            nc.gpsimd.dma_start(out=tri, in_=tr_d)
            ones = wp.tile([1, 64], f32r, tag="ones")
            nc.gpsimd.dma_start(out=ones, in_=on_d)

            kt_sb = [ktp.tile([128, T], bf16, name=f"kt{m}", tag=f"kt{m}")
                     for m in range(NMT)]
            v_sb = [vp.tile([128, VW], bf16, name=f"v{t}", tag=f"v{t}")
                    for t in range(T // 128)]
            for t in range(T // 128):
                nc.gpsimd.dma_start(
                    out=v_sb[t].rearrange("p (h e) -> p h e", h=NHL)[:, :, Dh:Dh + 1],
                    in_=vo_d.rearrange("p (h e) -> p h e", e=1))

            # ======== fine-grained projection / out-projection units,
            # consumed as PE filler between attention groups ========
            store = {}

            def load_x(ti):
                t0 = ti * TCH
                st = store[ti] = {}
                if ti == 0:
                    st["xt"] = xin0
                else:
                    xt = st["xt"] = [
                        xp.tile([128, TCH], bf16, name=f"xt{c}", tag=f"xt{c}")
                        for c in range(NCC)]
                    for c in range(NCC):
                        nc.sync.dma_start(
                            out=xt[c],
                            in_=xT_d[c * 128:(c + 1) * 128, t0:t0 + TCH])
                st["qt"] = [None] * NMT
                st["ps"] = {}

            def qk_units(ti):
                t0 = ti * TCH
                st = store[ti]

                def u_pq(m, half):
                    def f():
                        if half == 0:
                            pq = st["ps"]["q", m] = psM.tile(
                                [128, TCH], f32, tag="mm", name="pq")
                            for c in range(4):
                                mm(pq, wq_sb[c][:, m * 128:(m + 1) * 128],
                                   st["xt"][c], c == 0, False)
                        else:
                            pq = st["ps"].pop(("q", m))
                            for c in range(4, NCC):
                                mm(pq, wq_sb[c][:, m * 128:(m + 1) * 128],
                                   st["xt"][c], False, c == NCC - 1)
                            qtm = qtp.tile([128, TCH], bf16, name=f"qt{m}",
                                           tag=f"qt{m}")
                            st["qt"][m] = qtm
                            nc.vector.tensor_scalar_add(qtm, pq, bq_sb[m])
                    return f
                for m in range(NMT):
                    yield u_pq(m, 0)
                    yield u_pq(m, 1)

                def u_pk(m, half):
                    def f():
                        if half == 0:
                            pk = st["ps"]["k", m] = psM.tile(
                                [128, TCH], f32, tag="mm", name="pk")
                            for c in range(4):
                                mm(pk, wk_sb[c][:, m * 128:(m + 1) * 128],
                                   st["xt"][c], c == 0, False)
                        else:
                            pk = st["ps"].pop(("k", m))
                            for c in range(4, NCC):
                                mm(pk, wk_sb[c][:, m * 128:(m + 1) * 128],
                                   st["xt"][c], False, c == NCC - 1)
                            nc.vector.tensor_scalar_add(
                                kt_sb[m][:, t0:t0 + TCH], pk, bk_sb[m])
                    return f
                for m in range(NMT):
                    yield u_pk(m, 0)
                    yield u_pk(m, 1)

            def v_units(ti):
                t0 = ti * TCH
                st = store[ti]

                def u_pv(s, half):
                    def f():
                        if half == 0:
                            pv = st["ps"]["v", s] = psM.tile(
                                [128, HG], f32, tag="mm", name="pv")
                            for c in range(4):
                                mm(pv, st["xt"][c][:, s * 128:(s + 1) * 128],
                                   wv_sb[c], c == 0, False)
                        else:
                            pv = st["ps"].pop(("v", s))
                            for c in range(4, NCC):
                                mm(pv, st["xt"][c][:, s * 128:(s + 1) * 128],
                                   wv_sb[c], False, c == NCC - 1)
                            vt = v_sb[(t0 + s * 128) // 128]
                            nc.vector.tensor_add(
                                vt.rearrange("p (h e) -> p h e", h=NHL)[:, :, 0:Dh],
                                pv.rearrange("p (h d) -> p h d", h=NHL),
                                bv_bc.rearrange("p (h d) -> p h d", h=NHL))
                    return f
                for s in range(4):
                    yield u_pv(s, 0)
                    yield u_pv(s, 1)

            def outproj_units(qc, tail=False):
                ot = store[qc]["ot"]

                def u_py(s, n):
                    def f():
                        py = psM.tile([128, TCH], f32, tag="mm", name="py")
                        for m in range(NMT):
                            mm(py, ot[m][:, s * 128:(s + 1) * 128],
                               wo_sb[m][:, n * TCH:(n + 1) * TCH],
                               m == 0, m == NMT - 1)
                        yt = yst.tile([128, TCH], bf16, tag="yst", name="yt")
                        if tail and n == 1:
                            nc.scalar.copy(yt, py)
                        else:
                            nc.vector.tensor_copy(yt, py)
                        nc.sync.dma_start(
                            out=y_d[qc * TCH + s * 128:qc * TCH + (s + 1) * 128,
                                    n * TCH:(n + 1) * TCH],
                            in_=yt)
                    return f
                for s in range(4):
                    for n in range(2):
                        yield u_py(s, n)

            from collections import deque

            def merge_fillers(a, b):
                # spread b's units evenly through a's
                a, b = list(a), list(b)
                if not b:
                    return deque(a)
                out = deque()
                r = max(1, len(a) // (len(b) + 1))
                ai = 0
                for u in b:
                    out.extend(a[ai:ai + r])
                    ai += r
                    out.append(u)
                out.extend(a[ai:])
                return out

            deferred_muls = []

            def emit_deferred():
                while deferred_muls:
                    dst, src, rb = deferred_muls.pop(0)
                    nc.vector.tensor_mul(dst, src, rb)

            # chunk-0 projections up front, Q/K/V interleaved so all three
            # DMA queues' weight arrivals feed the PE concurrently
            load_x(0)
            qk0 = list(qk_units(0))   # [q(0,0),q(0,1),...,k(0,0),k(0,1),...]
            v0 = list(v_units(0))
            for m in range(NMT):
                for u in (qk0[2 * m], qk0[2 * m + 1],
                          qk0[8 + 2 * m], qk0[8 + 2 * m + 1],
                          v0[2 * m], v0[2 * m + 1]):
                    u()
            units = deque()

            for ti in range(NTC):
                while units:          # finish chunk ti's projections
                    units.popleft()()
                # filler plan: qc0 <- QKV(1); qc1 <- QKV(2);
                # qc2 <- QK(3) + outproj(0); qc3 <- V(3) + outproj(1) +
                # outproj(2)
                if ti + 1 < NTC:
                    load_x(ti + 1)    # start the x DMAs ahead of the fillers
                if ti <= 1:
                    units = deque(list(qk_units(ti + 1)) +
                                  list(v_units(ti + 1)))
                elif ti == 2:
                    op0 = list(outproj_units(0))
                    store["op0_tail"] = op0[4:]
                    units = merge_fillers(qk_units(3), op0[:4])
                else:
                    # ot(3) writes reuse ot(0)'s pool slot, so the deferred
                    # outproj(0) remnants must drain in the first steps
                    units = deque(store["op0_tail"] + list(v_units(3)) +
                                  list(merge_fillers(outproj_units(1),
                                                     outproj_units(2))))

                qc = ti
                nkt = 4 * (qc + 1)
                ngrp = nkt // 2
                qt = store[ti]["qt"]
                ot = store[ti]["ot"] = [
                    otp.tile([128, TCH], bf16, name=f"ot{m}", tag=f"ot{m}")
                    for m in range(NMT)]
                nsteps = NHL * ngrp
                step = 0
                # short chunks (few groups per head) interleave head PAIRS so
                # the S->exp->PV pipeline has twice the depth to fill
                if ngrp <= 6:
                    sched = []
                    for pr in range(NHL // 2):
                        for grp in range(ngrp):
                            sched.append((2 * pr, grp))
                            sched.append((2 * pr + 1, grp))
                else:
                    sched = [(h, grp) for h in range(NHL)
                             for grp in range(ngrp)]
                hstate = {}
                for h, grp in sched:
                    hp, base = h // 2, (h % 2) * 64
                    if grp == 0:
                        if h % 2 == 0 or ngrp > 4:
                            emit_deferred()
                        hstate[h] = {
                            "po": psO.tile([128, TCH], f32, tag="o",
                                           name="po"),
                            "pend": None,
                        }
                    st_h = hstate[h]
                    po = st_h["po"]
                    qth = qt[hp][base:base + 64, :]
                    kth = kt_sb[hp][base:base + 64, :]
                    k0, k1 = 2 * grp, 2 * grp + 1
                    # query offset per k-block (diagonal narrowing)
                    offs = [max(0, 128 * (k - (nkt - 4))) for k in (k0, k1)]
                    ps = psS.tile([128, 2 * TCH], f32, tag="sT", name="ps")
                    for slot, k in enumerate((k0, k1)):
                        off = offs[slot]
                        c0 = slot * TCH
                        mm(ps[:, c0:c0 + TCH - off],
                           kth[:, k * 128:(k + 1) * 128],
                           qth[:, off:TCH], True, True)
                    es = esp.tile([128, 2 * TCH], bf16, tag="es", name="es")
                    if offs[1] == 0:            # fully non-diagonal group
                        rngs = [(0, 2 * TCH)]
                    elif offs[0] == 0:          # dj=0: contiguous [0:896]
                        rngs = [(0, TCH + (TCH - offs[1]))]
                    else:                       # dj=1: two valid ranges
                        rngs = [(0, TCH - offs[0]),
                                (TCH, TCH + (TCH - offs[1]))]
                    for a, b in rngs:
                        nc.scalar.activation(es[:, a:b], ps[:, a:b],
                                             AF.Exp, scale=0.125)
                    if k0 >= nkt - 4:           # diagonal: mask leading tri
                        for slot in (0, 1):
                            c0 = slot * TCH
                            nc.vector.tensor_mul(
                                es[:, c0:c0 + 128], es[:, c0:c0 + 128], tri)
                    if st_h["pend"] is not None:
                        for (kk, oo, cc0, pes) in st_h["pend"]:
                            mm(po[0:Dh + 1, oo:TCH],
                               v_sb[kk][:, h * 65:(h + 1) * 65],
                               pes[:, cc0:cc0 + TCH - oo], kk == 0, False)
                    st_h["pend"] = [(k, offs[sl], sl * TCH, es)
                                    for sl, k in enumerate((k0, k1))]
                    # PE filler: keep the PE dense while ACT churns exps
                    step += 1
                    nfill = -(-len(units) // max(1, nsteps - step + 1))
                    if ti == NTC - 1 and step <= 6:
                        # drain deferred outproj(0) + chunk-3 V-projection
                        # ahead of their first consumers (ot(3) slot reuse,
                        # PV of k-blocks 12-15 at steps 7-8)
                        nfill = max(nfill, 2)
                    for _ in range(min(nfill, len(units))):
                        units.popleft()()
                    if grp < ngrp - 1:
                        continue
                    # ---- last group of head h: flush PV, normalize ----
                    for (kk, oo, cc0, pes) in st_h["pend"]:
                        mm(po[0:Dh + 1, oo:TCH],
                           v_sb[kk][:, h * 65:(h + 1) * 65],
                           pes[:, cc0:cc0 + TCH - oo], kk == 0, kk == nkt - 1)
                    # normalize: copy the denominator row to SBUF, DRAM-bounce
                    # it across 64 partitions, fast-reciprocal on the full
                    # [64,512] (reciprocal_approx_fast silently corrupts on HW
                    # for single-partition base-64 PSUM reads), then a mul
                    # deferred so the DMA latency never blocks the DVE queue
                    rpt = rp.tile([1, TCH], f32r, tag="rp", name="rpt")
                    nc.vector.tensor_copy(rpt, po[Dh:Dh + 1, :])
                    rbs = rp.tile([64, TCH], f32, tag="rbs", name="rbs")
                    if ti == NTC - 1 and h >= NHL - 2:
                        # tail: broadcast via PE (no DMA round-trip latency)
                        rb = psM.tile([128, TCH], f32, tag="mm", name="rb")
                        mm(rb[0:64, :], ones, rpt, True, True)
                        nc.vector.reciprocal_approx_fast(rbs, rb[0:64, :])
                    else:
                        idx = qc * NHL + h
                        row = rb_d[idx:idx + 1, :]
                        nc.sync.dma_start(out=row, in_=rpt.bitcast(f32))
                        rbr = rp.tile([64, TCH], f32, tag="rbr", name="rbr")
                        bsrc = bass.AP(tensor=row.tensor, offset=row.offset,
                                       ap=[[0, 64]] + list(row.ap)[1:])
                        nc.sync.dma_start(out=rbr, in_=bsrc)
                        nc.vector.reciprocal_approx_fast(rbs, rbr)
                    deferred_muls.append(
                        (ot[hp][base:base + 64, :], po[0:Dh, :], rbs))
                    if ti == NTC - 1 and h == NHL - 1:
                        emit_deferred()

            while units:
                units.popleft()()
            emit_deferred()
            for u in outproj_units(NTC - 1, tail=True):
                u()
            if _DEBUG_DUMP:
                for m in range(NMT):
                    nc.sync.dma_start(out=ktd_d[m], in_=kt_sb[m])
                    nc.sync.dma_start(out=otd_d[m], in_=store[3]["ot"][m])
                for t in range(T // 128):
                    nc.sync.dma_start(out=vd_d[t], in_=v_sb[t])
    nc.compile()
    return nc


def _get_nc():
    if "nc" not in _CACHE:
        _CACHE["nc"] = _build_nc()
    return _CACHE["nc"]


def _tri_np():
    import ml_dtypes
    p = np.arange(128)[:, None]
    u = np.arange(128)[None, :]
    return (u >= p).astype(ml_dtypes.bfloat16)


def _in_maps(x, w_qkv, b_qkv, w_out):
    import ml_dtypes
    bf = ml_dtypes.bfloat16
    tri = _tri_np()
    vones = np.ones((128, NHL), dtype=bf)
    maps = []
    for c in range(8):
        b, g = c // 2, c % 2
        s = g * HG
        maps.append({
            "xT": np.ascontiguousarray(x[b].T).astype(bf),
            "wq": np.ascontiguousarray(w_qkv[:, s:s + HG]).astype(bf),
            "wk": np.ascontiguousarray(w_qkv[:, C + s:C + s + HG]).astype(bf),
            "wv": np.ascontiguousarray(
                w_qkv[:, 2 * C + s:2 * C + s + HG]).astype(bf),
            "bq": np.ascontiguousarray(b_qkv[s:s + HG]),
            "bk": np.ascontiguousarray(b_qkv[C + s:C + s + HG]),
            "bv": np.ascontiguousarray(b_qkv[2 * C + s:2 * C + s + HG]),
            "wo": np.ascontiguousarray(w_out[s:s + HG, :]).astype(bf),
            "tri": tri,
            "vones": vones,
            "ones": np.ones((1, 64), dtype=np.float32),
        })
    return maps


def _run(x, w_qkv, b_qkv, w_out, b_out, trace=False, tmpdir=None):
    from concourse import bass_utils
    nc = _get_nc()
    maps = _in_maps(x, w_qkv, b_qkv, w_out)
    # the device occasionally reports a transient unrecoverable-exec error
    # right after a reset; one retry clears it
    last = None
    ys = None
    for attempt in range(4):
        try:
            res = bass_utils.run_bass_kernel_spmd(
                nc, maps, core_ids=list(range(8)), trace=trace, tmpdir=tmpdir)
        except Exception as e:
            last = e
            if attempt == 3:
                raise
            continue
        ys = [np.asarray(res.results[c]["y"], dtype=np.float32)
              for c in range(8)]
        # a wedged device occasionally returns garbage without erroring;
        # the true output is O(1), so sanity-check and retry
        if all(np.isfinite(y).all() and np.abs(y).max() < 1e3 for y in ys):
            break
    out = np.stack([ys[2 * b] + ys[2 * b + 1] for b in range(B)])
    out += np.asarray(b_out, dtype=np.float32)[None, None, :]
    return out.astype(np.float32), res


def kernel(x, w_qkv, b_qkv, w_out, b_out):
    x = np.asarray(x, dtype=np.float32)
    w_qkv = np.asarray(w_qkv, dtype=np.float32)
    b_qkv = np.asarray(b_qkv, dtype=np.float32)
    w_out = np.asarray(w_out, dtype=np.float32)
    b_out = np.asarray(b_out, dtype=np.float32)
    out, _ = _run(x, w_qkv, b_qkv, w_out, b_out, trace=False)
    return out
